# revision 10
# baseline (speedup 1.0000x reference)
"""Trainium2 Bass kernel: monomials x^a y^b z^c (a+b+c <= 3) for N=2M points.

Data-parallel across 8 NeuronCores; each core gets N/8 = 250k points padded
to 128*F*T. The trivial columns (1, x, y, z) are assembled host-side; the
device computes only the 16 degree>=2 monomials, minimizing HBM write
traffic (the binding roofline: ~358 GB/s per core).

Per tile of 128 x F points:
  in-tile  it [P, F, 3]  (point-major interleaved x,y,z; contiguous load)
  out-tile ot [P, F, 16] (point-major; contiguous store)
Device cols: 0:x2 1:xy 2:xz 3:y2 4:yz 5:z2
             6:x3 7:x2y 8:x2z 9:xy2 10:xyz 11:xz2 12:y3 13:y2z 14:yz2 15:z3
DVE (fused, step-0 broadcast in0): deg2 = x*(x,y,z)->0:3, y*(y,z)->3:5,
  z*z->5; deg3 = x*cols0:6->6:12, y*cols3:6->12:15, z*col5->15.
ACT: issues out-DMAs. SP: in-DMAs, just-in-time (front-loading all inputs
delays the output stream: the input queue has strict priority on the SDMA
engines).

Raw bass (no Tile): this walrus rejects >1 sync-wait per instruction, so all
waits are standalone wait_ge ops. Every tile has its own input slot and
sem; output slots are BO-deep with per-slot sems (one DMA in flight per sem
keeps 16*n waits unambiguous).
"""

import sys
from contextlib import ExitStack

if "/opt/trn_rl_repo" not in sys.path:
    sys.path.insert(0, "/opt/trn_rl_repo")

import numpy as np
import concourse.bass as bass
import concourse.mybir as mybir
from concourse.bass_utils import run_bass_kernel_spmd

P = 128
K = 20
KD = 16  # device-computed columns (degree >= 2)
N_TOTAL = 2_000_000
N_CORES = 8
N_CORE = N_TOTAL // N_CORES  # 250_000
F = 245
T = 8
BO = 3
N_PAD = P * F * T  # 250_880

AF = mybir.ActivationFunctionType
F32 = mybir.dt.float32
BF16 = mybir.dt.bfloat16


def build(nc: bass.Bass, n_pts: int, f: int, bo: int = BO) -> bass.Bass:
    t_total = n_pts // (P * f)
    assert t_total * P * f == n_pts

    v = nc.declare_dram_parameter("vectors", [n_pts, 3], F32, isOutput=False)
    o = nc.declare_dram_parameter("out", [n_pts, KD], BF16, isOutput=True)
    vr = v.rearrange("(t p f) c -> t p (f c)", p=P, f=f)
    orr = o.rearrange("(t p f) k -> t p (f k)", p=P, f=f)

    with ExitStack() as ctx:
        itb = ctx.enter_context(nc.sbuf_tensor("itb", [P, t_total * f * 3], F32))
        otb = ctx.enter_context(nc.sbuf_tensor("otb", [P, bo * f * KD], BF16))
        s_in = [ctx.enter_context(nc.semaphore(f"s_in{i}")) for i in range(t_total)]
        s_out = [ctx.enter_context(nc.semaphore(f"s_out{i}")) for i in range(bo)]
        s_v = ctx.enter_context(nc.semaphore("s_v"))
        s_g = ctx.enter_context(nc.semaphore("s_g"))
        block = ctx.enter_context(nc.Block(no_gpsimd_drain=True))

        def it_view(t):
            return itb.ap()[:, t * f * 3 : (t + 1) * f * 3].rearrange(
                "p (f c) -> p f c", c=3
            )

        def ot_flat(s):
            return otb.ap()[:, s * f * KD : (s + 1) * f * KD]

        def ot_view(s):
            return ot_flat(s).rearrange("p (f k) -> p f k", k=KD)

        @block.sync
        def _(sync):
            # Front-load all input DMAs: the input queue has strict priority
            # over the output queue on the SDMA engines, so interleaving
            # punches holes in the output stream. Serialized streams both
            # run at the HBM ceiling; ins finish before the first out needs
            # the engines.
            for t in range(t_total):
                sync.dma_start(
                    out=itb.ap()[:, t * f * 3 : (t + 1) * f * 3], in_=vr[t]
                ).then_inc(s_in[t], 16)

        @block.gpsimd
        def _(gpsimd):
            # Degree-2 products on GpSimd so DVE (degree-3) is not the wall.
            for t in range(t_total):
                s = t % bo
                n_use = t // bo
                itv = it_view(t)
                otv = ot_view(s)
                x = itv[:, :, 0:1]
                y = itv[:, :, 1:2]
                z = itv[:, :, 2:3]
                gpsimd.wait_ge(s_in[t], 16)
                if t >= bo:
                    gpsimd.wait_ge(s_out[s], 16 * n_use)
                nc.gpsimd.tensor_mul(
                    otv[:, :, 0:3], x.broadcast_to([P, f, 3]), itv[:, :, 0:3]
                )
                nc.gpsimd.tensor_mul(
                    otv[:, :, 3:5], y.broadcast_to([P, f, 2]), itv[:, :, 1:3]
                )
                nc.gpsimd.tensor_mul(otv[:, :, 5:6], z, z).then_inc(s_g, 1)

        @block.vector
        def _(vector):
            for t in range(t_total):
                s = t % bo
                n_use = t // bo  # completed uses of this out slot
                itv = it_view(t)
                otv = ot_view(s)
                x = itv[:, :, 0:1]
                y = itv[:, :, 1:2]
                z = itv[:, :, 2:3]
                vector.wait_ge(s_in[t], 16)
                if t >= bo:
                    # WAR: out-DMA of the tile previously in this slot done
                    vector.wait_ge(s_out[s], 16 * n_use)
                # RAW: deg3 reads GpSimd's deg2 columns.
                vector.wait_ge(s_g, t + 1)
                nc.vector.tensor_mul(
                    otv[:, :, 6:12], x.broadcast_to([P, f, 6]), otv[:, :, 0:6]
                )
                nc.vector.tensor_mul(
                    otv[:, :, 12:15], y.broadcast_to([P, f, 3]), otv[:, :, 3:6]
                )
                nc.vector.tensor_mul(otv[:, :, 15:16], z, otv[:, :, 5:6]).then_inc(
                    s_v, 1
                )

        @block.scalar
        def _(scalar):
            for t in range(t_total):
                s = t % bo
                scalar.wait_ge(s_v, t + 1)
                scalar.dma_start(out=orr[t], in_=ot_flat(s)).then_inc(s_out[s], 16)
            for s in range(bo):
                uses = len([t for t in range(t_total) if t % bo == s])
                if uses:
                    scalar.wait_ge(s_out[s], 16 * uses)

    return nc


_CACHE: dict[str, object] = {}


def _get_nc() -> bass.Bass:
    if "nc" not in _CACHE:
        nc = bass.Bass()
        build(nc, N_PAD, F, BO)
        _CACHE["nc"] = nc
    return _CACHE["nc"]  # type: ignore[return-value]


def run_spmd(in_maps, trace=False, **kw):
    return run_bass_kernel_spmd(
        _get_nc(), in_maps, core_ids=list(range(N_CORES)), trace=trace, **kw
    )


def make_in_maps(vectors: np.ndarray):
    vectors = np.ascontiguousarray(np.asarray(vectors, dtype=np.float32))
    assert vectors.shape == (N_TOTAL, 3)
    shards = vectors.reshape(N_CORES, N_CORE, 3)
    in_maps = []
    for i in range(N_CORES):
        buf = np.zeros((N_PAD, 3), dtype=np.float32)
        buf[:N_CORE] = shards[i]
        in_maps.append({"vectors": buf})
    return in_maps


def kernel(vectors: np.ndarray) -> np.ndarray:
    vec32 = np.ascontiguousarray(np.asarray(vectors, dtype=np.float32))
    res = run_spmd(make_in_maps(vec32))
    out = np.empty((N_TOTAL, K), dtype=np.float32)
    out[:, 0] = 1.0
    out[:, 1:4] = vec32  # degree-1 monomials are the input, exactly
    for i in range(N_CORES):
        out[i * N_CORE : (i + 1) * N_CORE, 4:] = np.asarray(
            res.results[i]["out"][:N_CORE], dtype=np.float32
        )
    return out
